# revision 30
# baseline (speedup 1.0000x reference)
"""Trainium2 Bass kernel for a 2-layer ViT (local banded MHA + global MHA, CLS head).

Contract: kernel(**inputs) takes the FULL fp32 inputs (as produced by
setup_inputs()) and returns the FULL [64, 1000] fp32 output. Internally the
batch (64) is sharded 8-ways across NeuronCores (data parallel); parameters are
replicated. Self-contained: shapes/sharding hardcoded.

Math notes:
 - activations held TRANSPOSED on chip: [D=768 (6 x 128 partitions), Ntok]
   with the 257 tokens padded to 264 columns (pads are masked/ignored).
 - local banded attention (radius 1): scores computed as S^T[k, q] per
   128-token k-chunk against a 130-wide q window around the diagonal; the
   attention-value matmul accumulates the overlapping q-windows into one PSUM
   tile (opened by a cheap ident x zeros start=True matmul, then pure
   has_written accumulates). NOTE: start=True matmuls at a non-zero free
   offset of a PSUM bank crash the exec unit - accumulation groups must be
   opened at offset 0 covering the full region.
 - softmax denominators for a head pair land in one [2, NPAD] PSUM tile via
   2-column selector matmuls (accumulated across the overlapping q-windows),
   1/z via reciprocal_approx_fast (~5x cheaper than DVE reciprocal, and on 2
   partitions instead of 2 single-lane [1, N] ops), broadcast back across the
   pair partitions with a single [2,128]-selector matmul.
 - LayerNorm 1 is batched across all 8 per-core batches: sums/sumsq collect
   into one [40, NPAD] PSUM tile via 40-column selector matmuls (sumsq rows
   at partitions 32..39 - engine partition offsets must be 32-aligned), the
   stats chain runs on [8, NPAD] tiles, and per-batch mean/rstd broadcasts
   are [8,128]-selector matmuls. Batching also cuts Exp<->Sqrt activation
   table swaps (1283ns each) from 18 to ~4 per run.
 - layer-2 computes K/V for all tokens but Q/attention/output only for the
   CLS token (the only row the model head consumes).
 - weights/activations bf16 on-chip, accumulation fp32 in PSUM, LN stats
   fp32. fp8 (DoubleRowSwInterleave) was tried and is 10% faster but the
   quantization noise (~7.5% rel err, weights dominate) blows the 2e-2 gate.
"""

import numpy as np
import ml_dtypes
from contextlib import ExitStack

BF16 = ml_dtypes.bfloat16

B, NCORES, BPC = 64, 8, 8
IMAGE, PATCH, GRID = 224, 14, 16
NPATCH, N, NPAD = 256, 257, 264
D, NH, HD, E, NCLS = 768, 12, 64, 2304, 1000
DC = D // 128            # 6 d-chunks
KP, KC = 98, 2           # patch-pixel contraction chunks: 196 = 2*98
SCALE = 1.0 / np.sqrt(HD)
NEG = -1e30
# k-chunks over tokens: (0:128, 128:256, 256:264); q-window per k-chunk
KCH = [(0, 128), (128, 128), (256, 8)]
QWIN = [(0, 130), (127, 130), (255, 9)]

_CACHE = {}


def _indh():
    ind = np.zeros((DC, 128, NH), np.float32)
    for dc in range(DC):
        for p in range(128):
            ind[dc, p, (128 * dc + p) // HD] = 1.0
    return ind.astype(BF16)


def _masks2():
    m = np.asarray(_masks(), np.float32)
    return np.ascontiguousarray(np.repeat(m[:, :, None, :], 2, axis=2))


def _colsel():
    m = np.zeros((128, 16, 40), np.float32)
    for j in range(8):
        m[:, j, j] = 1.0
        m[:, 8 + j, 32 + j] = 1.0
    return m.astype(BF16)


def _bsel():
    m = np.zeros((8, 8, 128), np.float32)
    for b in range(8):
        m[b, b, :] = 1.0
    return m.astype(BF16)


def _nsel2():
    m = np.zeros((2, 128), np.float32)
    m[0, 0:64] = 1.0
    m[1, 64:128] = 1.0
    return m.astype(BF16)


def _masks():
    m = np.full((3, 128, 130), NEG, np.float32)
    for c, ((k0, kn), (q0, qn)) in enumerate(zip(KCH, QWIN)):
        for kl in range(kn):
            gk = k0 + kl
            if gk > 256:
                continue
            for j in range(qn):
                gq = q0 + j
                if abs(gk - gq) <= 1 or (gq > 256 and gk <= 256):
                    m[c, kl, j] = 0.0
    return m


def build_nc(debug=False):
    import concourse.bacc as bacc
    import concourse.tile as tile
    from concourse import mybir
    import concourse.bass as bass

    f32, bf16 = mybir.dt.float32, mybir.dt.bfloat16
    AF, ALU = mybir.ActivationFunctionType, mybir.AluOpType

    nc = bacc.Bacc("TRN2", target_bir_lowering=False, debug=False)

    # ---- DRAM I/O ----
    d_pt = nc.dram_tensor("patchesT", [BPC, KC, KP, NPAD], bf16, kind="ExternalInput")
    d_wpT = nc.dram_tensor("wpT", [KP, KC, D], bf16, kind="ExternalInput")
    d_bp = nc.dram_tensor("bp", [DC, 128, NPAD], f32, kind="ExternalInput")
    d_wqkvT_l = nc.dram_tensor("wqkvT_l", [DC, 128, E], bf16, kind="ExternalInput")
    d_woT_l = nc.dram_tensor("woT_l", [DC, 128, D], bf16, kind="ExternalInput")
    d_wqkvT_g = nc.dram_tensor("wqkvT_g", [DC, 128, E], bf16, kind="ExternalInput")
    d_woT_g = nc.dram_tensor("woT_g", [DC, 128, D], bf16, kind="ExternalInput")
    d_wclsT = nc.dram_tensor("wclsT", [DC, 128, NCLS], bf16, kind="ExternalInput")
    d_mask = nc.dram_tensor("maskp2", [3, 128, 2, 130], f32, kind="ExternalInput")
    d_ident = nc.dram_tensor("ident", [128, 128], bf16, kind="ExternalInput")
    d_indh = nc.dram_tensor("indh", [DC, 128, NH], bf16, kind="ExternalInput")
    d_colsel = nc.dram_tensor("colsel", [128, 16, 40], bf16, kind="ExternalInput")
    d_nsel2 = nc.dram_tensor("nsel2", [2, 128], bf16, kind="ExternalInput")
    d_bsel = nc.dram_tensor("bsel", [8, 8, 128], bf16, kind="ExternalInput")
    d_bqkv_l = nc.dram_tensor("bqkv_l", [E], f32, kind="ExternalInput")
    d_bo_l = nc.dram_tensor("bo_l", [D], f32, kind="ExternalInput")
    d_bqkv_g = nc.dram_tensor("bqkv_g", [E], f32, kind="ExternalInput")
    d_bo_g = nc.dram_tensor("bo_g", [D], f32, kind="ExternalInput")
    d_g1 = nc.dram_tensor("g1", [D], f32, kind="ExternalInput")
    d_be1 = nc.dram_tensor("be1", [D], f32, kind="ExternalInput")
    d_g2 = nc.dram_tensor("g2", [D], f32, kind="ExternalInput")
    d_be2 = nc.dram_tensor("be2", [D], f32, kind="ExternalInput")
    d_bcls = nc.dram_tensor("b_cls", [NCLS], f32, kind="ExternalInput")
    d_out = nc.dram_tensor("logits", [BPC, NCLS], f32, kind="ExternalOutput")
    dbg = {}
    if debug:
        for nm, shp in [("dbg_tok", [DC, 128, NPAD]), ("dbg_qk", [12, 128, NPAD]),
                        ("dbg_av", [DC, 128, NPAD]), ("dbg_x1", [DC, 128, NPAD]),
                        ("dbg_local", [DC, 128, NPAD]), ("dbg_kg", [DC, 128, NPAD]),
                        ("dbg_sg", [1, NPAD]), ("dbg_ag", [1, D])]:
            dbg[nm] = nc.dram_tensor(nm, shp, f32, kind="ExternalOutput")

    with tile.TileContext(nc) as tc, ExitStack() as ctx:
        konst = ctx.enter_context(tc.tile_pool(name="konst", bufs=1))
        acts = ctx.enter_context(tc.tile_pool(name="acts", bufs=2))
        small = ctx.enter_context(tc.tile_pool(name="small", bufs=4))
        lnp = ctx.enter_context(tc.tile_pool(name="lnp", bufs=1))
        ps_mm = ctx.enter_context(tc.tile_pool(name="ps_mm", bufs=3, space="PSUM"))
        ps_pair = ctx.enter_context(tc.tile_pool(name="ps_pair", bufs=2, space="PSUM"))
        ps_v = ctx.enter_context(tc.tile_pool(name="ps_v", bufs=1, space="PSUM"))
        ps_row = ctx.enter_context(tc.tile_pool(name="ps_row", bufs=1, space="PSUM"))

        # ---- persistent SBUF ----
        wpT = konst.tile([KP, KC, D], bf16)
        nc.sync.dma_start(wpT, d_wpT.ap())
        wqkv_l = konst.tile([128, DC, E], bf16)
        wo_l = konst.tile([128, DC, D], bf16)
        wqkv_g = konst.tile([128, DC, E], bf16)
        wo_g = konst.tile([128, DC, D], bf16)
        wcls = konst.tile([128, DC, NCLS], bf16)
        bp = konst.tile([128, DC, NPAD], f32)
        for d in range(DC):
            nc.sync.dma_start(wqkv_l[:, d, :], d_wqkvT_l.ap()[d])
            nc.sync.dma_start(wo_l[:, d, :], d_woT_l.ap()[d])
            nc.sync.dma_start(wqkv_g[:, d, :], d_wqkvT_g.ap()[d])
            nc.sync.dma_start(wo_g[:, d, :], d_woT_g.ap()[d])
            nc.sync.dma_start(wcls[:, d, :], d_wclsT.ap()[d])
            nc.sync.dma_start(bp[:, d, :], d_bp.ap()[d])
        mask2 = konst.tile([128, 3, 2, 130], f32)
        for c in range(3):
            nc.sync.dma_start(mask2[:, c, :, :], d_mask.ap()[c])
        colsel = konst.tile([128, 16, 40], bf16)
        nc.sync.dma_start(colsel, d_colsel.ap())
        nsel2 = konst.tile([2, 128], bf16)
        nc.sync.dma_start(nsel2, d_nsel2.ap())
        bsel = konst.tile([8, 8, 128], bf16)
        nc.sync.dma_start(bsel, d_bsel.ap())
        ident = konst.tile([128, 128], bf16)
        nc.sync.dma_start(ident, d_ident.ap())
        indh = konst.tile([128, DC, NH], bf16)
        for d in range(DC):
            nc.sync.dma_start(indh[:, d, :], d_indh.ap()[d])
        zrow768 = konst.tile([1, D], bf16)
        bqkv_l_c = konst.tile([128, 18], f32)
        nc.sync.dma_start(bqkv_l_c, d_bqkv_l.ap().rearrange("(j p) -> p j", p=128))
        bqkv_g_c = konst.tile([128, 18], f32)
        nc.sync.dma_start(bqkv_g_c, d_bqkv_g.ap().rearrange("(j p) -> p j", p=128))
        bo_l_c = konst.tile([128, DC], f32)
        nc.sync.dma_start(bo_l_c, d_bo_l.ap().rearrange("(j p) -> p j", p=128))
        bo_g_c = konst.tile([128, DC], f32)
        nc.sync.dma_start(bo_g_c, d_bo_g.ap().rearrange("(j p) -> p j", p=128))
        g1_c = konst.tile([128, DC], f32)
        nc.sync.dma_start(g1_c, d_g1.ap().rearrange("(j p) -> p j", p=128))
        be1_c = konst.tile([128, DC], f32)
        nc.sync.dma_start(be1_c, d_be1.ap().rearrange("(j p) -> p j", p=128))
        g2_c = konst.tile([128, DC], f32)
        nc.sync.dma_start(g2_c, d_g2.ap().rearrange("(j p) -> p j", p=128))
        be2_c = konst.tile([128, DC], f32)
        nc.sync.dma_start(be2_c, d_be2.ap().rearrange("(j p) -> p j", p=128))
        bcls_r = konst.tile([BPC, NCLS], f32)
        nc.sync.dma_start(
            bcls_r,
            bass.AP(tensor=d_bcls, offset=0, ap=[[0, BPC], [1, NCLS]]),
        )
        zeros264 = konst.tile([128, NPAD], bf16)
        nc.vector.memset(zeros264, 0.0)
        ones_col = konst.tile([128, 1], bf16)
        nc.vector.memset(ones_col, 1.0)
        ones_row = konst.tile([1, 128], bf16)
        nc.vector.memset(ones_row, 1.0)
        zrow = konst.tile([1, NPAD], bf16)
        nc.vector.memset(zrow, 0.0)
        nc.vector.memset(zrow768, 0.0)
        epsc = konst.tile([128, 1], f32)
        nc.vector.memset(epsc, 1e-5)

        LOCAL = konst.tile([128, BPC, DC, NPAD], bf16)   # post-LN1, all batches
        AGROWS = konst.tile([BPC, D], bf16)              # global attn out rows
        QCLS = konst.tile([128, DC, BPC], f32)          # global q for CLS

        def evict(dst, src, bias=None, scale=1.0):
            if bias is None:
                nc.scalar.activation(dst, src, AF.Copy, scale=scale)
            else:
                nc.scalar.activation(dst, src, AF.Identity, bias=bias, scale=scale)

        # ================= pass 1: per batch through LN1 =================
        for b in range(BPC):
            pt = acts.tile([KP, KC, NPAD], bf16, tag="pt")
            for k in range(KC):
                nc.sync.dma_start(pt[:, k, :], d_pt.ap()[b, k])
            tokT = acts.tile([128, DC, NPAD], bf16, tag="tokT")
            for d in range(DC):
                ps = ps_mm.tile([128, NPAD], f32, tag="mm")
                for k in range(KC):
                    nc.tensor.matmul(ps, lhsT=wpT[:, k, bass.ts(d, 128)],
                                     rhs=pt[:, k, :], start=(k == 0), stop=(k == KC - 1))
                nc.vector.tensor_add(tokT[:, d, :], ps, bp[:, d, :])
            if debug and b == 0:
                for d in range(DC):
                    nc.gpsimd.dma_start(dbg["dbg_tok"].ap()[d], tokT[:, d, :])

            # Q^T,K^T (e-chunks 0..11), layer 1
            qkT = acts.tile([128, 12, NPAD], bf16, tag="qkT")
            for e in range(12):
                ps = ps_mm.tile([128, NPAD], f32, tag="mm")
                for d in range(DC):
                    nc.tensor.matmul(ps, lhsT=wqkv_l[:, d, bass.ts(e, 128)],
                                     rhs=tokT[:, d, :], start=(d == 0), stop=(d == DC - 1))
                evict(qkT[:, e, :], ps, bias=bqkv_l_c[:, e:e + 1])
            if debug and b == 0:
                for e in range(12):
                    nc.gpsimd.dma_start(dbg["dbg_qk"].ap()[e], qkT[:, e, :])

            # V token-major, layer 1
            v0 = acts.tile([128, D], bf16, tag="v0")
            v1 = acts.tile([128, D], bf16, tag="v1")
            v2 = acts.tile([8, D], bf16, tag="v2")
            for t, vt in ((0, v0), (1, v1), (2, v2)):
                k0, kn = KCH[t]
                ps = ps_v.tile([128, D], f32, tag="vps")
                for n0, nn in ((0, 512), (512, 256)):
                    for d in range(DC):
                        nc.tensor.matmul(ps[:kn, n0:n0 + nn],
                                         lhsT=tokT[:, d, k0:k0 + kn],
                                         rhs=wqkv_l[:, d, 2 * D + n0:2 * D + n0 + nn],
                                         start=(d == 0), stop=(d == DC - 1))
                evict(vt, ps[:kn, :])

            # local banded attention
            AVT = acts.tile([128, DC, NPAD], bf16, tag="AVT")
            for hp in range(6):          # head pairs
                pair = ps_pair.tile([128, NPAD], f32, tag="pair")
                nc.tensor.matmul(pair, lhsT=ident, rhs=zeros264,
                                 start=True, stop=False, skip_group_check=True)
                zz = ps_row.tile([2, NPAD], f32, tag="row")
                nc.tensor.matmul(zz, lhsT=ident[:, 0:2], rhs=zeros264,
                                 start=True, stop=False, skip_group_check=True)
                ets = []
                for c in range(3):
                    (k0, kn), (q0, qn) = KCH[c], QWIN[c]
                    et = small.tile([128, 2, 130], bf16, tag="et")
                    for hh in range(2):
                        r0 = 64 * hh
                        sps = ps_mm.tile([128, 130], f32, tag="mm")
                        nc.tensor.matmul(sps[:kn, :qn],
                                         lhsT=qkT[r0:r0 + 64, 6 + hp, k0:k0 + kn],
                                         rhs=qkT[r0:r0 + 64, hp, q0:q0 + qn],
                                         start=True, stop=True)
                        nc.vector.scalar_tensor_tensor(
                            out=et[:kn, hh, :qn], in0=sps[:kn, :qn], scalar=SCALE,
                            in1=mask2[:kn, c, hh, :qn], op0=ALU.mult, op1=ALU.add)
                        nc.scalar.activation(et[:kn, hh, :qn], et[:kn, hh, :qn], AF.Exp)
                    ets.append(et)
                for hh in range(2):
                    h = 2 * hp + hh
                    r0 = 64 * hh
                    for c in range(3):
                        (k0, kn), (q0, qn) = KCH[c], QWIN[c]
                        et = ets[c]
                        nc.tensor.matmul(pair[r0:r0 + 64, q0:q0 + qn],
                                         lhsT=vt_sel(v0, v1, v2, c)[:kn, h * HD:(h + 1) * HD],
                                         rhs=et[:kn, hh, :qn],
                                         start=False, stop=(hh == 1 and c == 2),
                                         skip_group_check=True)
                        nc.tensor.matmul(zz[0:2, q0:q0 + qn],
                                         lhsT=colsel[:kn, hh, 0:2],
                                         rhs=et[:kn, hh, :qn],
                                         start=False, stop=(hh == 1 and c == 2),
                                         skip_group_check=True)
                zrf = small.tile([2, NPAD], f32, tag="zrf")
                nc.vector.reciprocal_approx_fast(out=zrf, in_=zz)
                zrb = small.tile([2, NPAD], bf16, tag="zrb")
                evict(zrb, zrf)
                normps = ps_mm.tile([128, NPAD], f32, tag="mm")
                nc.tensor.matmul(normps, lhsT=nsel2, rhs=zrb, start=True, stop=True)
                norm_sb = small.tile([128, NPAD], bf16, tag="normsb")
                evict(norm_sb, normps)
                nc.vector.tensor_mul(AVT[:, hp, :], pair, norm_sb)
            if debug and b == 0:
                for d in range(DC):
                    nc.gpsimd.dma_start(dbg["dbg_av"].ap()[d], AVT[:, d, :])

            # out-projection layer 1 -> x1, stored into LOCAL (LN'd in place later)
            for e in range(DC):
                ps = ps_mm.tile([128, NPAD], f32, tag="mm")
                for f in range(DC):
                    nc.tensor.matmul(ps, lhsT=wo_l[:, f, bass.ts(e, 128)],
                                     rhs=AVT[:, f, :], start=(f == 0), stop=(f == DC - 1))
                evict(LOCAL[:, b, e, :], ps, bias=bo_l_c[:, e:e + 1])
            if debug and b == 0:
                for d in range(DC):
                    nc.gpsimd.dma_start(dbg["dbg_x1"].ap()[d], LOCAL[:, b, d, :])

        # ============ LayerNorm 1, batched across all 8 batches ============
        stats = ps_row.tile([40, NPAD], f32, tag="row")
        nc.tensor.matmul(stats, lhsT=ident[:, 0:40], rhs=zeros264,
                         start=True, stop=False, skip_group_check=True)
        for b in range(BPC):
            for d in range(DC):
                sq = acts.tile([128, NPAD], bf16, tag="sq")
                nc.vector.tensor_mul(sq, LOCAL[:, b, d, :], LOCAL[:, b, d, :])
                nc.tensor.matmul(stats, lhsT=colsel[:, b, 0:40], rhs=LOCAL[:, b, d, :],
                                 start=False, stop=False, skip_group_check=True)
                nc.tensor.matmul(stats, lhsT=colsel[:, 8 + b, 0:40], rhs=sq,
                                 start=False, stop=(b == BPC - 1 and d == DC - 1),
                                 skip_group_check=True)
        st8 = lnp.tile([8, NPAD], f32)
        evict(st8, stats[0:8, :])
        mu2 = lnp.tile([8, NPAD], f32)
        nc.vector.scalar_tensor_tensor(out=mu2, in0=st8, scalar=1.0 / (D * D),
                                       in1=st8, op0=ALU.mult, op1=ALU.mult)
        var8 = lnp.tile([8, NPAD], f32)
        nc.vector.scalar_tensor_tensor(out=var8, in0=stats[32:40, :], scalar=1.0 / D,
                                       in1=mu2, op0=ALU.mult, op1=ALU.subtract)
        sd8 = lnp.tile([8, NPAD], f32)
        nc.scalar.activation(sd8, var8, AF.Sqrt, bias=epsc[0:8, :])
        rstd8f = lnp.tile([8, NPAD], f32)
        nc.vector.reciprocal_approx_fast(out=rstd8f, in_=sd8)
        rstd8 = lnp.tile([8, NPAD], bf16)
        evict(rstd8, rstd8f)
        mu8 = lnp.tile([8, NPAD], bf16)
        evict(mu8, st8, scale=1.0 / D)
        for b in range(BPC):
            bmu = ps_mm.tile([128, NPAD], f32, tag="mm")
            nc.tensor.matmul(bmu, lhsT=bsel[:, b, :], rhs=mu8, start=True, stop=True)
            brs = ps_mm.tile([128, NPAD], f32, tag="mm")
            nc.tensor.matmul(brs, lhsT=bsel[:, b, :], rhs=rstd8, start=True, stop=True)
            for d in range(DC):
                t1 = acts.tile([128, NPAD], bf16, tag="t1")
                nc.vector.tensor_sub(t1, LOCAL[:, b, d, :], bmu)
                t2 = acts.tile([128, NPAD], bf16, tag="t2")
                nc.vector.tensor_mul(t2, t1, brs)
                nc.scalar.activation(LOCAL[:, b, d, :], t2, AF.Identity,
                                     bias=be1_c[:, d:d + 1], scale=g1_c[:, d:d + 1])
        if debug:
            for d in range(DC):
                nc.gpsimd.dma_start(dbg["dbg_local"].ap()[d], LOCAL[:, 0, d, :])

        # ================= pass 2: global q for CLS (all batches) =================
        for e in range(DC):
            ps = ps_row.tile([128, BPC], f32, tag="row")
            for d in range(DC):
                nc.tensor.matmul(ps, lhsT=wqkv_g[:, d, bass.ts(e, 128)],
                                 rhs=LOCAL[:, :, d, 0], start=(d == 0), stop=(d == DC - 1))
            evict(QCLS[:, e, :], ps, bias=bqkv_g_c[:, e:e + 1])

        # ================= pass 3: global attention per batch =================
        for b in range(BPC):
            kgT = acts.tile([128, 6, NPAD], bf16, tag="AVT")
            for e in range(DC):
                ps = ps_mm.tile([128, NPAD], f32, tag="mm")
                for d in range(DC):
                    nc.tensor.matmul(ps, lhsT=wqkv_g[:, d, D + 128 * e:D + 128 * (e + 1)],
                                     rhs=LOCAL[:, b, d, :], start=(d == 0), stop=(d == DC - 1))
                evict(kgT[:, e, :], ps, bias=bqkv_g_c[:, 6 + e:7 + e])
            if debug and b == 0:
                for d in range(DC):
                    nc.gpsimd.dma_start(dbg["dbg_kg"].ap()[d], kgT[:, d, :])
            vg0 = acts.tile([128, D], bf16, tag="v0")
            vg1 = acts.tile([128, D], bf16, tag="v1")
            vg2 = acts.tile([8, D], bf16, tag="v2")
            for t, vt in ((0, vg0), (1, vg1), (2, vg2)):
                k0, kn = KCH[t]
                ps = ps_v.tile([128, D], f32, tag="vps")
                for n0, nn in ((0, 512), (512, 256)):
                    for d in range(DC):
                        nc.tensor.matmul(ps[:kn, n0:n0 + nn],
                                         lhsT=LOCAL[:, b, d, k0:k0 + kn],
                                         rhs=wqkv_g[:, d, 2 * D + n0:2 * D + n0 + nn],
                                         start=(d == 0), stop=(d == DC - 1))
                evict(vt, ps[:kn, :])

            # per-head scores as columns: SCOL[k, h] per k-chunk via the
            # block-diagonal-expanded q (Qblk[d, h] = q[d] iff d in head h).
            qblk = small.tile([128, DC, NH], bf16, tag="qblk")
            for d in range(DC):
                nc.vector.tensor_scalar_mul(qblk[:, d, :], indh[:, d, :],
                                            QCLS[:, d, b:b + 1])
            scps = ps_mm.tile([128, 3 * NH], f32, tag="mm")
            nc.tensor.matmul(scps, lhsT=ones_row, rhs=zrow[:, 0:3 * NH],
                             start=True, stop=False, skip_group_check=True)
            for c in range(3):
                k0, kn = KCH[c]
                for d in range(DC):
                    nc.tensor.matmul(scps[:kn, NH * c:NH * (c + 1)],
                                     lhsT=kgT[:, d, k0:k0 + kn], rhs=qblk[:, d, :],
                                     start=False, stop=(c == 2 and d == DC - 1),
                                     skip_group_check=True)
            ecol = small.tile([128, 3 * NH], bf16, tag="ecol")
            # chunk 2 holds pad k-tokens 257..263 in rows 1..7: zero the block
            # first, then exp only the real row 0 (WAW keeps the order).
            nc.vector.memset(ecol[0:8, 2 * NH:3 * NH], 0.0)
            for c in range(3):
                kn = KCH[c][1] if c < 2 else 1
                nc.scalar.activation(ecol[:kn, NH * c:NH * (c + 1)],
                                     scps[:kn, NH * c:NH * (c + 1)], AF.Exp, scale=SCALE)
            zg = ps_mm.tile([1, NH], f32, tag="mm")
            for c in range(3):
                kn = KCH[c][1] if c < 2 else 1
                nc.tensor.matmul(zg, lhsT=ones_col[:kn, :],
                                 rhs=ecol[:kn, NH * c:NH * (c + 1)],
                                 start=(c == 0), stop=(c == 2), skip_group_check=True)
            rzg = small.tile([1, NH], f32, tag="rowb")
            nc.vector.reciprocal_approx_fast(out=rzg, in_=zg)
            rzgb = small.tile([1, NH], bf16, tag="rowa")
            evict(rzgb, rzg)
            bzps = ps_mm.tile([128, NH], f32, tag="mm")
            nc.tensor.matmul(bzps, lhsT=ones_row, rhs=rzgb, start=True, stop=True)
            rzbc = small.tile([128, NH], bf16, tag="rzbc")
            evict(rzbc, bzps)
            ecoln = small.tile([128, 3 * NH], bf16, tag="ecoln")
            for c in range(3):
                kn = 128 if c < 2 else 8
                nc.vector.tensor_mul(ecoln[:kn, NH * c:NH * (c + 1)],
                                     ecol[:kn, NH * c:NH * (c + 1)], rzbc[:kn, :])

            agps = ps_v.tile([1, D], f32, tag="vps")
            for n0, nn in ((0, 512), (512, 256)):
                nc.tensor.matmul(agps[0:1, n0:n0 + nn], lhsT=ones_row[0:1, 0:1],
                                 rhs=zrow768[:, n0:n0 + nn],
                                 start=True, stop=False, skip_group_check=True)
            for h in range(NH):
                for c, vt in ((0, vg0), (1, vg1), (2, vg2)):
                    kn = KCH[c][1]
                    nc.tensor.matmul(agps[0:1, h * HD:(h + 1) * HD],
                                     lhsT=ecoln[:kn, NH * c + h:NH * c + h + 1],
                                     rhs=vt[:kn, h * HD:(h + 1) * HD],
                                     start=False, stop=(h == NH - 1 and c == 2),
                                     skip_group_check=True)
            if debug and b == 0:
                ag_sb = konst.tile([1, D], f32)
                evict(ag_sb, agps)
                nc.gpsimd.dma_start(dbg["dbg_ag"].ap(), ag_sb)
            agrow = small.tile([1, D], bf16, tag="rowa")
            evict(agrow, agps)
            nc.sync.dma_start(AGROWS[b:b + 1, :], agrow)

        # ================= tail: wo_g, LN2, classifier =================
        attg = konst.tile([128, DC, BPC], bf16)
        for d in range(DC):
            tps = ps_mm.tile([128, BPC], bf16, tag="mm")
            nc.tensor.transpose(tps, AGROWS[:, bass.ts(d, 128)], ident[0:BPC, 0:BPC])
            evict(attg[:, d, :], tps)
        ogt = konst.tile([128, DC, BPC], bf16)
        for e in range(DC):
            ps = ps_row.tile([128, BPC], f32, tag="row")
            for f in range(DC):
                nc.tensor.matmul(ps, lhsT=wo_g[:, f, bass.ts(e, 128)],
                                 rhs=attg[:, f, :], start=(f == 0), stop=(f == DC - 1))
            evict(ogt[:, e, :], ps, bias=bo_g_c[:, e:e + 1])
        # LN2
        ps_s = ps_row.tile([1, BPC], f32, tag="row")
        ps_q = ps_mm.tile([1, BPC], f32, tag="mm")
        for d in range(DC):
            sq2 = small.tile([128, BPC], bf16, tag="sq2")
            nc.vector.tensor_mul(sq2, ogt[:, d, :], ogt[:, d, :])
            nc.tensor.matmul(ps_s, lhsT=ones_col, rhs=ogt[:, d, :],
                             start=(d == 0), stop=(d == DC - 1))
            nc.tensor.matmul(ps_q, lhsT=ones_col, rhs=sq2,
                             start=(d == 0), stop=(d == DC - 1))
        mu = small.tile([1, BPC], bf16, tag="rowa")
        evict(mu, ps_s, scale=1.0 / D)
        muf = small.tile([1, BPC], f32, tag="rowb")
        evict(muf, ps_s, scale=1.0 / D)
        m2 = small.tile([1, BPC], f32, tag="rowa")
        nc.vector.tensor_mul(m2, muf, muf)
        var = small.tile([1, BPC], f32, tag="rowb")
        nc.vector.scalar_tensor_tensor(out=var, in0=ps_q, scalar=1.0 / D,
                                       in1=m2, op0=ALU.mult, op1=ALU.subtract)
        sd = small.tile([1, BPC], f32, tag="rowa")
        nc.scalar.activation(sd, var, AF.Sqrt, bias=epsc[0:1, :])
        rstd = small.tile([1, BPC], f32, tag="rowb")
        nc.vector.reciprocal_approx_fast(out=rstd, in_=sd)
        rstd_b = small.tile([1, BPC], bf16, tag="rowa")
        evict(rstd_b, rstd)
        bmu = ps_mm.tile([128, BPC], f32, tag="mm")
        nc.tensor.matmul(bmu, lhsT=ones_row, rhs=mu, start=True, stop=True)
        brs = ps_mm.tile([128, BPC], f32, tag="mm")
        nc.tensor.matmul(brs, lhsT=ones_row, rhs=rstd_b, start=True, stop=True)
        lng = konst.tile([128, DC, BPC], bf16)
        for d in range(DC):
            t1 = small.tile([128, BPC], bf16, tag="t1s")
            nc.vector.tensor_sub(t1, ogt[:, d, :], bmu)
            t2 = small.tile([128, BPC], bf16, tag="t2s")
            nc.vector.tensor_mul(t2, t1, brs)
            nc.scalar.activation(lng[:, d, :], t2, AF.Identity,
                                 bias=be2_c[:, d:d + 1], scale=g2_c[:, d:d + 1])
        # classifier
        outsb = konst.tile([BPC, NCLS], f32)
        for n0, nn in ((0, 512), (512, NCLS - 512)):
            ps = ps_mm.tile([BPC, 512], f32, tag="mm")
            for d in range(DC):
                nc.tensor.matmul(ps[:, :nn], lhsT=lng[:, d, :],
                                 rhs=wcls[:, d, n0:n0 + nn],
                                 start=(d == 0), stop=(d == DC - 1))
            nc.vector.tensor_add(outsb[:, n0:n0 + nn], ps[:, :nn], bcls_r[:, n0:n0 + nn])
        nc.sync.dma_start(d_out.ap(), outsb)

    nc.compile()
    return nc


def vt_sel(v0, v1, v2, c):
    return (v0, v1, v2)[c]


def prep_inputs(inputs):
    """numpy-only host prep: shard x; transpose/bcast/pack parameters."""
    f = lambda k: np.asarray(inputs[k], np.float32)
    x = f("x")
    pat = x[:, 0].reshape(B, GRID, PATCH, GRID, PATCH)
    pat = pat.transpose(0, 2, 4, 1, 3).reshape(B, PATCH * PATCH, NPATCH)
    patchesT = np.zeros((B, KP * KC, NPAD), np.float32)
    patchesT[:, :, 1:N] = pat
    patchesT = patchesT.reshape(B, KC, KP, NPAD).astype(BF16)

    wpT = f("w_patch").T.reshape(KC, KP, D).transpose(1, 0, 2).astype(BF16)

    pos = f("pos_embedding")[0]              # [257, 768]
    bp = np.zeros((D, NPAD), np.float32)
    bp[:, 1:N] = f("b_patch")[:, None] + pos[1:].T
    bp[:, 0] = f("cls_token")[0, 0] + pos[0]
    bp = bp.reshape(DC, 128, NPAD)

    shared = {
        "wpT": wpT,
        "bp": bp,
        "wqkvT_l": f("wqkv_l").T.reshape(DC, 128, E).astype(BF16),
        "woT_l": f("wo_l").T.reshape(DC, 128, D).astype(BF16),
        "wqkvT_g": f("wqkv_g").T.reshape(DC, 128, E).astype(BF16),
        "woT_g": f("wo_g").T.reshape(DC, 128, D).astype(BF16),
        "wclsT": f("w_cls").T.reshape(DC, 128, NCLS).astype(BF16),
        "maskp2": _masks2(), "colsel": _colsel(), "nsel2": _nsel2(), "bsel": _bsel(),
        "ident": np.eye(128, dtype=np.float32).astype(BF16),
        "indh": _indh(),
        "bqkv_l": f("bqkv_l"), "bo_l": f("bo_l"),
        "bqkv_g": f("bqkv_g"), "bo_g": f("bo_g"),
        "g1": f("g1"), "be1": f("be1"), "g2": f("g2"), "be2": f("be2"),
        "b_cls": f("b_cls"),
    }
    in_maps = []
    for c in range(NCORES):
        m = dict(shared)
        m["patchesT"] = patchesT[c * BPC:(c + 1) * BPC]
        in_maps.append(m)
    return in_maps


def kernel(**inputs) -> np.ndarray:
    if "nc" not in _CACHE:
        _CACHE["nc"] = build_nc(debug=False)
    nc = _CACHE["nc"]
    from concourse.bass_utils import run_bass_kernel_spmd
    in_maps = prep_inputs(inputs)
    res = run_bass_kernel_spmd(nc, in_maps, core_ids=list(range(NCORES)))
    return np.concatenate([r["logits"] for r in res.results], axis=0).astype(np.float32)



# revision 31
# speedup vs baseline: 1.0431x; 1.0431x over previous
"""Trainium2 Bass kernel for a 2-layer ViT (local banded MHA + global MHA, CLS head).

Contract: kernel(**inputs) takes the FULL fp32 inputs (as produced by
setup_inputs()) and returns the FULL [64, 1000] fp32 output. Internally the
batch (64) is sharded 8-ways across NeuronCores (data parallel); parameters are
replicated. Self-contained: shapes/sharding hardcoded.

Math notes:
 - activations held TRANSPOSED on chip: [D=768 (6 x 128 partitions), Ntok]
   with the 257 tokens padded to 264 columns (pads are masked/ignored).
 - local banded attention (radius 1): scores computed as S^T[k, q] per
   128-token k-chunk against a 130-wide q window around the diagonal; the
   attention-value matmul accumulates the overlapping q-windows into one PSUM
   tile (opened by a cheap ident x zeros start=True matmul, then pure
   has_written accumulates). NOTE: start=True matmuls at a non-zero free
   offset of a PSUM bank crash the exec unit - accumulation groups must be
   opened at offset 0 covering the full region.
 - softmax denominators for a head pair land in one [2, NPAD] PSUM tile via
   2-column selector matmuls (accumulated across the overlapping q-windows),
   1/z via reciprocal_approx_fast (~5x cheaper than DVE reciprocal, and on 2
   partitions instead of 2 single-lane [1, N] ops), broadcast back across the
   pair partitions with a single [2,128]-selector matmul.
 - LayerNorm 1 is batched across all 8 per-core batches: sums/sumsq collect
   into one [40, NPAD] PSUM tile via 40-column selector matmuls (sumsq rows
   at partitions 32..39 - engine partition offsets must be 32-aligned), the
   stats chain runs on [8, NPAD] tiles, and per-batch mean/rstd broadcasts
   are [8,128]-selector matmuls. Batching also cuts Exp<->Sqrt activation
   table swaps (1283ns each) from 18 to ~4 per run.
 - layer-2 computes K/V for all tokens but Q/attention/output only for the
   CLS token (the only row the model head consumes).
 - weights/activations bf16 on-chip, accumulation fp32 in PSUM, LN stats
   fp32. fp8 (DoubleRowSwInterleave) was tried and is 10% faster but the
   quantization noise (~7.5% rel err, weights dominate) blows the 2e-2 gate.
"""

import numpy as np
import ml_dtypes
from contextlib import ExitStack

BF16 = ml_dtypes.bfloat16

B, NCORES, BPC = 64, 8, 8
IMAGE, PATCH, GRID = 224, 14, 16
NPATCH, N, NPAD = 256, 257, 264
D, NH, HD, E, NCLS = 768, 12, 64, 2304, 1000
DC = D // 128            # 6 d-chunks
KP, KC = 98, 2           # patch-pixel contraction chunks: 196 = 2*98
SCALE = 1.0 / np.sqrt(HD)
NEG = -1e30
# k-chunks over tokens: (0:128, 128:256, 256:264); q-window per k-chunk
KCH = [(0, 128), (128, 128), (256, 8)]
QWIN = [(0, 130), (127, 130), (255, 9)]

_CACHE = {}


def _indh():
    ind = np.zeros((DC, 128, NH), np.float32)
    for dc in range(DC):
        for p in range(128):
            ind[dc, p, (128 * dc + p) // HD] = 1.0
    return ind.astype(BF16)


def _masks2():
    m = np.asarray(_masks(), np.float32)
    return np.ascontiguousarray(np.repeat(m[:, :, None, :], 2, axis=2))


def _colsel():
    m = np.zeros((128, 16, 40), np.float32)
    for j in range(8):
        m[:, j, j] = 1.0
        m[:, 8 + j, 32 + j] = 1.0
    return m.astype(BF16)


def _bsel():
    m = np.zeros((8, 8, 128), np.float32)
    for b in range(8):
        m[b, b, :] = 1.0
    return m.astype(BF16)


def _nsel2():
    m = np.zeros((2, 128), np.float32)
    m[0, 0:64] = 1.0
    m[1, 64:128] = 1.0
    return m.astype(BF16)


def _masks():
    m = np.full((3, 128, 130), NEG, np.float32)
    for c, ((k0, kn), (q0, qn)) in enumerate(zip(KCH, QWIN)):
        for kl in range(kn):
            gk = k0 + kl
            if gk > 256:
                continue
            for j in range(qn):
                gq = q0 + j
                if abs(gk - gq) <= 1 or (gq > 256 and gk <= 256):
                    m[c, kl, j] = 0.0
    return m


def build_nc(debug=False):
    import concourse.bacc as bacc
    import concourse.tile as tile
    from concourse import mybir
    import concourse.bass as bass

    f32, bf16 = mybir.dt.float32, mybir.dt.bfloat16
    AF, ALU = mybir.ActivationFunctionType, mybir.AluOpType

    nc = bacc.Bacc("TRN2", target_bir_lowering=False, debug=False)

    # ---- DRAM I/O ----
    d_pt = nc.dram_tensor("patchesT", [BPC, KC, KP, NPAD], bf16, kind="ExternalInput")
    d_wpT = nc.dram_tensor("wpT", [KP, KC, D], bf16, kind="ExternalInput")
    d_bp = nc.dram_tensor("bp", [DC, 128, NPAD], f32, kind="ExternalInput")
    d_wqkvT_l = nc.dram_tensor("wqkvT_l", [DC, 128, E], bf16, kind="ExternalInput")
    d_woT_l = nc.dram_tensor("woT_l", [DC, 128, D], bf16, kind="ExternalInput")
    d_wqkvT_g = nc.dram_tensor("wqkvT_g", [DC, 128, E], bf16, kind="ExternalInput")
    d_woT_g = nc.dram_tensor("woT_g", [DC, 128, D], bf16, kind="ExternalInput")
    d_wclsT = nc.dram_tensor("wclsT", [DC, 128, NCLS], bf16, kind="ExternalInput")
    d_mask = nc.dram_tensor("maskp2", [3, 128, 2, 130], f32, kind="ExternalInput")
    d_ident = nc.dram_tensor("ident", [128, 128], bf16, kind="ExternalInput")
    d_indh = nc.dram_tensor("indh", [DC, 128, NH], bf16, kind="ExternalInput")
    d_colsel = nc.dram_tensor("colsel", [128, 16, 40], bf16, kind="ExternalInput")
    d_nsel2 = nc.dram_tensor("nsel2", [2, 128], bf16, kind="ExternalInput")
    d_bsel = nc.dram_tensor("bsel", [8, 8, 128], bf16, kind="ExternalInput")
    d_bqkv_l = nc.dram_tensor("bqkv_l", [E], f32, kind="ExternalInput")
    d_bo_l = nc.dram_tensor("bo_l", [D], f32, kind="ExternalInput")
    d_bqkv_g = nc.dram_tensor("bqkv_g", [E], f32, kind="ExternalInput")
    d_bo_g = nc.dram_tensor("bo_g", [D], f32, kind="ExternalInput")
    d_g1 = nc.dram_tensor("g1", [D], f32, kind="ExternalInput")
    d_be1 = nc.dram_tensor("be1", [D], f32, kind="ExternalInput")
    d_g2 = nc.dram_tensor("g2", [D], f32, kind="ExternalInput")
    d_be2 = nc.dram_tensor("be2", [D], f32, kind="ExternalInput")
    d_bcls = nc.dram_tensor("b_cls", [NCLS], f32, kind="ExternalInput")
    d_out = nc.dram_tensor("logits", [BPC, NCLS], f32, kind="ExternalOutput")
    dbg = {}
    if debug:
        for nm, shp in [("dbg_tok", [DC, 128, NPAD]), ("dbg_qk", [12, 128, NPAD]),
                        ("dbg_av", [DC, 128, NPAD]), ("dbg_x1", [DC, 128, NPAD]),
                        ("dbg_local", [DC, 128, NPAD]), ("dbg_kg", [DC, 128, NPAD]),
                        ("dbg_sg", [1, NPAD]), ("dbg_ag", [1, D])]:
            dbg[nm] = nc.dram_tensor(nm, shp, f32, kind="ExternalOutput")

    with tile.TileContext(nc) as tc, ExitStack() as ctx:
        konst = ctx.enter_context(tc.tile_pool(name="konst", bufs=1))
        acts = ctx.enter_context(tc.tile_pool(name="acts", bufs=2))
        small = ctx.enter_context(tc.tile_pool(name="small", bufs=4))
        lnp = ctx.enter_context(tc.tile_pool(name="lnp", bufs=1))
        ps_mm = ctx.enter_context(tc.tile_pool(name="ps_mm", bufs=3, space="PSUM"))
        ps_pair = ctx.enter_context(tc.tile_pool(name="ps_pair", bufs=2, space="PSUM"))
        ps_v = ctx.enter_context(tc.tile_pool(name="ps_v", bufs=1, space="PSUM"))
        ps_row = ctx.enter_context(tc.tile_pool(name="ps_row", bufs=1, space="PSUM"))

        # ---- persistent SBUF ----
        wpT = konst.tile([KP, KC, D], bf16)
        nc.sync.dma_start(wpT, d_wpT.ap())
        wqkv_l = konst.tile([128, DC, E], bf16)
        wo_l = konst.tile([128, DC, D], bf16)
        wqkv_g = konst.tile([128, DC, E], bf16)
        wo_g = konst.tile([128, DC, D], bf16)
        wcls = konst.tile([128, DC, NCLS], bf16)
        bp = konst.tile([128, DC, NPAD], f32)
        for d in range(DC):
            nc.sync.dma_start(wqkv_l[:, d, :], d_wqkvT_l.ap()[d])
            nc.sync.dma_start(wo_l[:, d, :], d_woT_l.ap()[d])
            nc.sync.dma_start(wqkv_g[:, d, :], d_wqkvT_g.ap()[d])
            nc.sync.dma_start(wo_g[:, d, :], d_woT_g.ap()[d])
            nc.sync.dma_start(wcls[:, d, :], d_wclsT.ap()[d])
            nc.sync.dma_start(bp[:, d, :], d_bp.ap()[d])
        mask2 = konst.tile([128, 3, 2, 130], f32)
        for c in range(3):
            nc.sync.dma_start(mask2[:, c, :, :], d_mask.ap()[c])
        colsel = konst.tile([128, 16, 40], bf16)
        nc.sync.dma_start(colsel, d_colsel.ap())
        nsel2 = konst.tile([2, 128], bf16)
        nc.sync.dma_start(nsel2, d_nsel2.ap())
        bsel = konst.tile([8, 8, 128], bf16)
        nc.sync.dma_start(bsel, d_bsel.ap())
        ident = konst.tile([128, 128], bf16)
        nc.sync.dma_start(ident, d_ident.ap())
        indh = konst.tile([128, DC, NH], bf16)
        for d in range(DC):
            nc.sync.dma_start(indh[:, d, :], d_indh.ap()[d])
        zrow768 = konst.tile([1, D], bf16)
        bqkv_l_c = konst.tile([128, 18], f32)
        nc.sync.dma_start(bqkv_l_c, d_bqkv_l.ap().rearrange("(j p) -> p j", p=128))
        bqkv_g_c = konst.tile([128, 18], f32)
        nc.sync.dma_start(bqkv_g_c, d_bqkv_g.ap().rearrange("(j p) -> p j", p=128))
        bo_l_c = konst.tile([128, DC], f32)
        nc.sync.dma_start(bo_l_c, d_bo_l.ap().rearrange("(j p) -> p j", p=128))
        bo_g_c = konst.tile([128, DC], f32)
        nc.sync.dma_start(bo_g_c, d_bo_g.ap().rearrange("(j p) -> p j", p=128))
        g1_c = konst.tile([128, DC], f32)
        nc.sync.dma_start(g1_c, d_g1.ap().rearrange("(j p) -> p j", p=128))
        be1_c = konst.tile([128, DC], f32)
        nc.sync.dma_start(be1_c, d_be1.ap().rearrange("(j p) -> p j", p=128))
        g2_c = konst.tile([128, DC], f32)
        nc.sync.dma_start(g2_c, d_g2.ap().rearrange("(j p) -> p j", p=128))
        be2_c = konst.tile([128, DC], f32)
        nc.sync.dma_start(be2_c, d_be2.ap().rearrange("(j p) -> p j", p=128))
        bcls_r = konst.tile([BPC, NCLS], f32)
        nc.sync.dma_start(
            bcls_r,
            bass.AP(tensor=d_bcls, offset=0, ap=[[0, BPC], [1, NCLS]]),
        )
        zeros264 = konst.tile([128, NPAD], bf16)
        nc.vector.memset(zeros264, 0.0)
        ones_col = konst.tile([128, 1], bf16)
        nc.vector.memset(ones_col, 1.0)
        ones_row = konst.tile([1, 128], bf16)
        nc.vector.memset(ones_row, 1.0)
        zrow = konst.tile([1, NPAD], bf16)
        nc.vector.memset(zrow, 0.0)
        nc.vector.memset(zrow768, 0.0)
        epsc = konst.tile([128, 1], f32)
        nc.vector.memset(epsc, 1e-5)

        LOCAL = konst.tile([128, BPC, DC, NPAD], bf16)   # post-LN1, all batches
        AGROWS = konst.tile([BPC, D], bf16)              # global attn out rows
        QCLS = konst.tile([128, DC, BPC], f32)          # global q for CLS

        def evict(dst, src, bias=None, scale=1.0):
            if bias is None:
                nc.scalar.activation(dst, src, AF.Copy, scale=scale)
            else:
                nc.scalar.activation(dst, src, AF.Identity, bias=bias, scale=scale)

        # ================= pass 1: per batch through LN1 =================
        for b in range(BPC):
            pt = acts.tile([KP, KC, NPAD], bf16, tag="pt")
            for k in range(KC):
                nc.sync.dma_start(pt[:, k, :], d_pt.ap()[b, k])
            tokT = acts.tile([128, DC, NPAD], bf16, tag="tokT")
            for d in range(DC):
                ps = ps_mm.tile([128, NPAD], f32, tag="mm")
                for k in range(KC):
                    nc.tensor.matmul(ps, lhsT=wpT[:, k, bass.ts(d, 128)],
                                     rhs=pt[:, k, :], start=(k == 0), stop=(k == KC - 1))
                nc.vector.tensor_add(tokT[:, d, :], ps, bp[:, d, :])
            if debug and b == 0:
                for d in range(DC):
                    nc.gpsimd.dma_start(dbg["dbg_tok"].ap()[d], tokT[:, d, :])

            # Q^T,K^T (e-chunks 0..11), layer 1
            qkT = acts.tile([128, 12, NPAD], bf16, tag="qkT")
            for e in range(12):
                ps = ps_mm.tile([128, NPAD], f32, tag="mm")
                for d in range(DC):
                    nc.tensor.matmul(ps, lhsT=wqkv_l[:, d, bass.ts(e, 128)],
                                     rhs=tokT[:, d, :], start=(d == 0), stop=(d == DC - 1))
                evict(qkT[:, e, :], ps, bias=bqkv_l_c[:, e:e + 1])
            if debug and b == 0:
                for e in range(12):
                    nc.gpsimd.dma_start(dbg["dbg_qk"].ap()[e], qkT[:, e, :])

            # V token-major, layer 1
            v0 = acts.tile([128, D], bf16, tag="v0")
            v1 = acts.tile([128, D], bf16, tag="v1")
            v2 = acts.tile([8, D], bf16, tag="v2")
            for t, vt in ((0, v0), (1, v1), (2, v2)):
                k0, kn = KCH[t]
                ps = ps_v.tile([128, D], f32, tag="vps")
                for n0, nn in ((0, 512), (512, 256)):
                    for d in range(DC):
                        nc.tensor.matmul(ps[:kn, n0:n0 + nn],
                                         lhsT=tokT[:, d, k0:k0 + kn],
                                         rhs=wqkv_l[:, d, 2 * D + n0:2 * D + n0 + nn],
                                         start=(d == 0), stop=(d == DC - 1))
                evict(vt, ps[:kn, :])

            # local banded attention
            AVT = acts.tile([128, DC, NPAD], bf16, tag="AVT")
            for hp in range(6):          # head pairs
                pair = ps_pair.tile([128, NPAD], f32, tag="pair")
                nc.tensor.matmul(pair, lhsT=ident, rhs=zeros264,
                                 start=True, stop=False, skip_group_check=True)
                zz = ps_row.tile([2, NPAD], f32, tag="row")
                nc.tensor.matmul(zz, lhsT=ident[:, 0:2], rhs=zeros264,
                                 start=True, stop=False, skip_group_check=True)
                ets = []
                for c in range(3):
                    (k0, kn), (q0, qn) = KCH[c], QWIN[c]
                    et = small.tile([128, 2, 130], bf16, tag="et")
                    for hh in range(2):
                        r0 = 64 * hh
                        sps = ps_mm.tile([128, 130], f32, tag="mm")
                        nc.tensor.matmul(sps[:kn, :qn],
                                         lhsT=qkT[r0:r0 + 64, 6 + hp, k0:k0 + kn],
                                         rhs=qkT[r0:r0 + 64, hp, q0:q0 + qn],
                                         start=True, stop=True)
                        nc.vector.scalar_tensor_tensor(
                            out=et[:kn, hh, :qn], in0=sps[:kn, :qn], scalar=SCALE,
                            in1=mask2[:kn, c, hh, :qn], op0=ALU.mult, op1=ALU.add)
                        nc.scalar.activation(et[:kn, hh, :qn], et[:kn, hh, :qn], AF.Exp)
                    ets.append(et)
                for hh in range(2):
                    h = 2 * hp + hh
                    r0 = 64 * hh
                    for c in range(3):
                        (k0, kn), (q0, qn) = KCH[c], QWIN[c]
                        et = ets[c]
                        nc.tensor.matmul(pair[r0:r0 + 64, q0:q0 + qn],
                                         lhsT=vt_sel(v0, v1, v2, c)[:kn, h * HD:(h + 1) * HD],
                                         rhs=et[:kn, hh, :qn],
                                         start=False, stop=(hh == 1 and c == 2),
                                         skip_group_check=True)
                        nc.tensor.matmul(zz[0:2, q0:q0 + qn],
                                         lhsT=colsel[:kn, hh, 0:2],
                                         rhs=et[:kn, hh, :qn],
                                         start=False, stop=(hh == 1 and c == 2),
                                         skip_group_check=True)
                zrf = small.tile([2, NPAD], f32, tag="zrf")
                nc.vector.reciprocal_approx_fast(out=zrf, in_=zz)
                zrb = small.tile([2, NPAD], bf16, tag="zrb")
                evict(zrb, zrf)
                normps = ps_pair.tile([128, NPAD], f32, tag="pair")
                nc.tensor.matmul(normps, lhsT=nsel2, rhs=zrb, start=True, stop=True)
                norm_sb = small.tile([128, NPAD], bf16, tag="normsb")
                evict(norm_sb, normps)
                nc.vector.tensor_mul(AVT[:, hp, :], pair, norm_sb)
            if debug and b == 0:
                for d in range(DC):
                    nc.gpsimd.dma_start(dbg["dbg_av"].ap()[d], AVT[:, d, :])

            # out-projection layer 1 -> x1, stored into LOCAL (LN'd in place later)
            for e in range(DC):
                ps = ps_mm.tile([128, NPAD], f32, tag="mm")
                for f in range(DC):
                    nc.tensor.matmul(ps, lhsT=wo_l[:, f, bass.ts(e, 128)],
                                     rhs=AVT[:, f, :], start=(f == 0), stop=(f == DC - 1))
                evict(LOCAL[:, b, e, :], ps, bias=bo_l_c[:, e:e + 1])
            if debug and b == 0:
                for d in range(DC):
                    nc.gpsimd.dma_start(dbg["dbg_x1"].ap()[d], LOCAL[:, b, d, :])

        # ============ LayerNorm 1, batched across all 8 batches ============
        stats = ps_row.tile([40, NPAD], f32, tag="row")
        nc.tensor.matmul(stats, lhsT=ident[:, 0:40], rhs=zeros264,
                         start=True, stop=False, skip_group_check=True)
        for b in range(BPC):
            for d in range(DC):
                sq = acts.tile([128, NPAD], bf16, tag="sq")
                nc.vector.tensor_mul(sq, LOCAL[:, b, d, :], LOCAL[:, b, d, :])
                nc.tensor.matmul(stats, lhsT=colsel[:, b, 0:40], rhs=LOCAL[:, b, d, :],
                                 start=False, stop=False, skip_group_check=True)
                nc.tensor.matmul(stats, lhsT=colsel[:, 8 + b, 0:40], rhs=sq,
                                 start=False, stop=(b == BPC - 1 and d == DC - 1),
                                 skip_group_check=True)
        st8 = lnp.tile([8, NPAD], f32)
        evict(st8, stats[0:8, :])
        mu2 = lnp.tile([8, NPAD], f32)
        nc.vector.scalar_tensor_tensor(out=mu2, in0=st8, scalar=1.0 / (D * D),
                                       in1=st8, op0=ALU.mult, op1=ALU.mult)
        var8 = lnp.tile([8, NPAD], f32)
        nc.vector.scalar_tensor_tensor(out=var8, in0=stats[32:40, :], scalar=1.0 / D,
                                       in1=mu2, op0=ALU.mult, op1=ALU.subtract)
        sd8 = lnp.tile([8, NPAD], f32)
        nc.scalar.activation(sd8, var8, AF.Sqrt, bias=epsc[0:8, :])
        rstd8f = lnp.tile([8, NPAD], f32)
        nc.vector.reciprocal_approx_fast(out=rstd8f, in_=sd8)
        rstd8 = lnp.tile([8, NPAD], bf16)
        evict(rstd8, rstd8f)
        mu8 = lnp.tile([8, NPAD], bf16)
        evict(mu8, st8, scale=1.0 / D)
        for b in range(BPC):
            bmu = ps_mm.tile([128, NPAD], f32, tag="mm")
            nc.tensor.matmul(bmu, lhsT=bsel[:, b, :], rhs=mu8, start=True, stop=True)
            brs = ps_mm.tile([128, NPAD], f32, tag="mm")
            nc.tensor.matmul(brs, lhsT=bsel[:, b, :], rhs=rstd8, start=True, stop=True)
            for d in range(DC):
                t1 = acts.tile([128, NPAD], bf16, tag="t1")
                nc.vector.tensor_sub(t1, LOCAL[:, b, d, :], bmu)
                t2 = acts.tile([128, NPAD], bf16, tag="t2")
                nc.vector.tensor_mul(t2, t1, brs)
                nc.scalar.activation(LOCAL[:, b, d, :], t2, AF.Identity,
                                     bias=be1_c[:, d:d + 1], scale=g1_c[:, d:d + 1])
        if debug:
            for d in range(DC):
                nc.gpsimd.dma_start(dbg["dbg_local"].ap()[d], LOCAL[:, 0, d, :])

        # ================= pass 2: global q for CLS (all batches) =================
        for e in range(DC):
            ps = ps_row.tile([128, BPC], f32, tag="row")
            for d in range(DC):
                nc.tensor.matmul(ps, lhsT=wqkv_g[:, d, bass.ts(e, 128)],
                                 rhs=LOCAL[:, :, d, 0], start=(d == 0), stop=(d == DC - 1))
            evict(QCLS[:, e, :], ps, bias=bqkv_g_c[:, e:e + 1])

        # ================= pass 3: global attention per batch =================
        for b in range(BPC):
            kgT = acts.tile([128, 6, NPAD], bf16, tag="AVT")
            for e in range(DC):
                ps = ps_mm.tile([128, NPAD], f32, tag="mm")
                for d in range(DC):
                    nc.tensor.matmul(ps, lhsT=wqkv_g[:, d, D + 128 * e:D + 128 * (e + 1)],
                                     rhs=LOCAL[:, b, d, :], start=(d == 0), stop=(d == DC - 1))
                evict(kgT[:, e, :], ps, bias=bqkv_g_c[:, 6 + e:7 + e])
            if debug and b == 0:
                for d in range(DC):
                    nc.gpsimd.dma_start(dbg["dbg_kg"].ap()[d], kgT[:, d, :])
            vg0 = acts.tile([128, D], bf16, tag="v0")
            vg1 = acts.tile([128, D], bf16, tag="v1")
            vg2 = acts.tile([8, D], bf16, tag="v2")
            for t, vt in ((0, vg0), (1, vg1), (2, vg2)):
                k0, kn = KCH[t]
                ps = ps_v.tile([128, D], f32, tag="vps")
                for n0, nn in ((0, 512), (512, 256)):
                    for d in range(DC):
                        nc.tensor.matmul(ps[:kn, n0:n0 + nn],
                                         lhsT=LOCAL[:, b, d, k0:k0 + kn],
                                         rhs=wqkv_g[:, d, 2 * D + n0:2 * D + n0 + nn],
                                         start=(d == 0), stop=(d == DC - 1))
                evict(vt, ps[:kn, :])

            # per-head scores as columns: SCOL[k, h] per k-chunk via the
            # block-diagonal-expanded q (Qblk[d, h] = q[d] iff d in head h).
            qblk = small.tile([128, DC, NH], bf16, tag="qblk")
            for d in range(DC):
                nc.vector.tensor_scalar_mul(qblk[:, d, :], indh[:, d, :],
                                            QCLS[:, d, b:b + 1])
            scps = ps_mm.tile([128, 3 * NH], f32, tag="mm")
            nc.tensor.matmul(scps, lhsT=ones_row, rhs=zrow[:, 0:3 * NH],
                             start=True, stop=False, skip_group_check=True)
            for c in range(3):
                k0, kn = KCH[c]
                for d in range(DC):
                    nc.tensor.matmul(scps[:kn, NH * c:NH * (c + 1)],
                                     lhsT=kgT[:, d, k0:k0 + kn], rhs=qblk[:, d, :],
                                     start=False, stop=(c == 2 and d == DC - 1),
                                     skip_group_check=True)
            ecol = small.tile([128, 3 * NH], bf16, tag="ecol")
            # chunk 2 holds pad k-tokens 257..263 in rows 1..7: zero the block
            # first, then exp only the real row 0 (WAW keeps the order).
            nc.vector.memset(ecol[0:8, 2 * NH:3 * NH], 0.0)
            for c in range(3):
                kn = KCH[c][1] if c < 2 else 1
                nc.scalar.activation(ecol[:kn, NH * c:NH * (c + 1)],
                                     scps[:kn, NH * c:NH * (c + 1)], AF.Exp, scale=SCALE)
            zg = ps_mm.tile([1, NH], f32, tag="mm")
            for c in range(3):
                kn = KCH[c][1] if c < 2 else 1
                nc.tensor.matmul(zg, lhsT=ones_col[:kn, :],
                                 rhs=ecol[:kn, NH * c:NH * (c + 1)],
                                 start=(c == 0), stop=(c == 2), skip_group_check=True)
            rzg = small.tile([1, NH], f32, tag="rowb")
            nc.vector.reciprocal_approx_fast(out=rzg, in_=zg)
            rzgb = small.tile([1, NH], bf16, tag="rowa")
            evict(rzgb, rzg)
            bzps = ps_mm.tile([128, NH], f32, tag="mm")
            nc.tensor.matmul(bzps, lhsT=ones_row, rhs=rzgb, start=True, stop=True)
            rzbc = small.tile([128, NH], bf16, tag="rzbc")
            evict(rzbc, bzps)
            ecoln = small.tile([128, 3 * NH], bf16, tag="ecoln")
            for c in range(3):
                kn = 128 if c < 2 else 8
                nc.vector.tensor_mul(ecoln[:kn, NH * c:NH * (c + 1)],
                                     ecol[:kn, NH * c:NH * (c + 1)], rzbc[:kn, :])

            agps = ps_v.tile([1, D], f32, tag="vps")
            for n0, nn in ((0, 512), (512, 256)):
                nc.tensor.matmul(agps[0:1, n0:n0 + nn], lhsT=ones_row[0:1, 0:1],
                                 rhs=zrow768[:, n0:n0 + nn],
                                 start=True, stop=False, skip_group_check=True)
            for h in range(NH):
                for c, vt in ((0, vg0), (1, vg1), (2, vg2)):
                    kn = KCH[c][1]
                    nc.tensor.matmul(agps[0:1, h * HD:(h + 1) * HD],
                                     lhsT=ecoln[:kn, NH * c + h:NH * c + h + 1],
                                     rhs=vt[:kn, h * HD:(h + 1) * HD],
                                     start=False, stop=(h == NH - 1 and c == 2),
                                     skip_group_check=True)
            if debug and b == 0:
                ag_sb = konst.tile([1, D], f32)
                evict(ag_sb, agps)
                nc.gpsimd.dma_start(dbg["dbg_ag"].ap(), ag_sb)
            agrow = small.tile([1, D], bf16, tag="rowa")
            evict(agrow, agps)
            nc.sync.dma_start(AGROWS[b:b + 1, :], agrow)

        # ================= tail: wo_g, LN2, classifier =================
        attg = konst.tile([128, DC, BPC], bf16)
        for d in range(DC):
            tps = ps_mm.tile([128, BPC], bf16, tag="mm")
            nc.tensor.transpose(tps, AGROWS[:, bass.ts(d, 128)], ident[0:BPC, 0:BPC])
            evict(attg[:, d, :], tps)
        ogt = konst.tile([128, DC, BPC], bf16)
        for e in range(DC):
            ps = ps_row.tile([128, BPC], f32, tag="row")
            for f in range(DC):
                nc.tensor.matmul(ps, lhsT=wo_g[:, f, bass.ts(e, 128)],
                                 rhs=attg[:, f, :], start=(f == 0), stop=(f == DC - 1))
            evict(ogt[:, e, :], ps, bias=bo_g_c[:, e:e + 1])
        # LN2
        ps_s = ps_row.tile([1, BPC], f32, tag="row")
        ps_q = ps_mm.tile([1, BPC], f32, tag="mm")
        for d in range(DC):
            sq2 = small.tile([128, BPC], bf16, tag="sq2")
            nc.vector.tensor_mul(sq2, ogt[:, d, :], ogt[:, d, :])
            nc.tensor.matmul(ps_s, lhsT=ones_col, rhs=ogt[:, d, :],
                             start=(d == 0), stop=(d == DC - 1))
            nc.tensor.matmul(ps_q, lhsT=ones_col, rhs=sq2,
                             start=(d == 0), stop=(d == DC - 1))
        mu = small.tile([1, BPC], bf16, tag="rowa")
        evict(mu, ps_s, scale=1.0 / D)
        muf = small.tile([1, BPC], f32, tag="rowb")
        evict(muf, ps_s, scale=1.0 / D)
        m2 = small.tile([1, BPC], f32, tag="rowa")
        nc.vector.tensor_mul(m2, muf, muf)
        var = small.tile([1, BPC], f32, tag="rowb")
        nc.vector.scalar_tensor_tensor(out=var, in0=ps_q, scalar=1.0 / D,
                                       in1=m2, op0=ALU.mult, op1=ALU.subtract)
        sd = small.tile([1, BPC], f32, tag="rowa")
        nc.scalar.activation(sd, var, AF.Sqrt, bias=epsc[0:1, :])
        rstd = small.tile([1, BPC], f32, tag="rowb")
        nc.vector.reciprocal_approx_fast(out=rstd, in_=sd)
        rstd_b = small.tile([1, BPC], bf16, tag="rowa")
        evict(rstd_b, rstd)
        bmu = ps_mm.tile([128, BPC], f32, tag="mm")
        nc.tensor.matmul(bmu, lhsT=ones_row, rhs=mu, start=True, stop=True)
        brs = ps_mm.tile([128, BPC], f32, tag="mm")
        nc.tensor.matmul(brs, lhsT=ones_row, rhs=rstd_b, start=True, stop=True)
        lng = konst.tile([128, DC, BPC], bf16)
        for d in range(DC):
            t1 = small.tile([128, BPC], bf16, tag="t1s")
            nc.vector.tensor_sub(t1, ogt[:, d, :], bmu)
            t2 = small.tile([128, BPC], bf16, tag="t2s")
            nc.vector.tensor_mul(t2, t1, brs)
            nc.scalar.activation(lng[:, d, :], t2, AF.Identity,
                                 bias=be2_c[:, d:d + 1], scale=g2_c[:, d:d + 1])
        # classifier
        outsb = konst.tile([BPC, NCLS], f32)
        for n0, nn in ((0, 512), (512, NCLS - 512)):
            ps = ps_mm.tile([BPC, 512], f32, tag="mm")
            for d in range(DC):
                nc.tensor.matmul(ps[:, :nn], lhsT=lng[:, d, :],
                                 rhs=wcls[:, d, n0:n0 + nn],
                                 start=(d == 0), stop=(d == DC - 1))
            nc.vector.tensor_add(outsb[:, n0:n0 + nn], ps[:, :nn], bcls_r[:, n0:n0 + nn])
        nc.sync.dma_start(d_out.ap(), outsb)

    nc.compile()
    return nc


def vt_sel(v0, v1, v2, c):
    return (v0, v1, v2)[c]


def prep_inputs(inputs):
    """numpy-only host prep: shard x; transpose/bcast/pack parameters."""
    f = lambda k: np.asarray(inputs[k], np.float32)
    x = f("x")
    pat = x[:, 0].reshape(B, GRID, PATCH, GRID, PATCH)
    pat = pat.transpose(0, 2, 4, 1, 3).reshape(B, PATCH * PATCH, NPATCH)
    patchesT = np.zeros((B, KP * KC, NPAD), np.float32)
    patchesT[:, :, 1:N] = pat
    patchesT = patchesT.reshape(B, KC, KP, NPAD).astype(BF16)

    wpT = f("w_patch").T.reshape(KC, KP, D).transpose(1, 0, 2).astype(BF16)

    pos = f("pos_embedding")[0]              # [257, 768]
    bp = np.zeros((D, NPAD), np.float32)
    bp[:, 1:N] = f("b_patch")[:, None] + pos[1:].T
    bp[:, 0] = f("cls_token")[0, 0] + pos[0]
    bp = bp.reshape(DC, 128, NPAD)

    shared = {
        "wpT": wpT,
        "bp": bp,
        "wqkvT_l": f("wqkv_l").T.reshape(DC, 128, E).astype(BF16),
        "woT_l": f("wo_l").T.reshape(DC, 128, D).astype(BF16),
        "wqkvT_g": f("wqkv_g").T.reshape(DC, 128, E).astype(BF16),
        "woT_g": f("wo_g").T.reshape(DC, 128, D).astype(BF16),
        "wclsT": f("w_cls").T.reshape(DC, 128, NCLS).astype(BF16),
        "maskp2": _masks2(), "colsel": _colsel(), "nsel2": _nsel2(), "bsel": _bsel(),
        "ident": np.eye(128, dtype=np.float32).astype(BF16),
        "indh": _indh(),
        "bqkv_l": f("bqkv_l"), "bo_l": f("bo_l"),
        "bqkv_g": f("bqkv_g"), "bo_g": f("bo_g"),
        "g1": f("g1"), "be1": f("be1"), "g2": f("g2"), "be2": f("be2"),
        "b_cls": f("b_cls"),
    }
    in_maps = []
    for c in range(NCORES):
        m = dict(shared)
        m["patchesT"] = patchesT[c * BPC:(c + 1) * BPC]
        in_maps.append(m)
    return in_maps


def kernel(**inputs) -> np.ndarray:
    if "nc" not in _CACHE:
        _CACHE["nc"] = build_nc(debug=False)
    nc = _CACHE["nc"]
    from concourse.bass_utils import run_bass_kernel_spmd
    in_maps = prep_inputs(inputs)
    res = run_bass_kernel_spmd(nc, in_maps, core_ids=list(range(NCORES)))
    return np.concatenate([r["logits"] for r in res.results], axis=0).astype(np.float32)

